# Initial kernel scaffold
#
"""APPNP propagation (10 iterations of h <- 0.9*A@h + 0.1*x) on 8 TRN2 NeuronCores.

Strategy (edge/row sharding + ELLPACK):
  - Nodes are permuted so rows are grouped by degree into blocks of 128; each
    core owns 98 blocks (12544 rows) chosen snake-wise so per-core edge counts
    balance. Per block, every row's neighbor list is padded to the block's max
    degree g, giving a regular [128, g] index/weight layout per block.
  - Per iteration, each core indirect-DMA-gathers the bf16 h rows of its
    edges' source nodes ([128, g, 64] tiles), multiplies by edge weights and
    segment-reduces on the vector engine, adds the residual, and AllGathers
    the bf16 h shards into the next iteration's gather table.
  - Output rows are written once (f32) on the last iteration and unpermuted
    on the host.
"""

import sys

sys.path.insert(0, "/opt/trn_rl_repo")

import numpy as np
import ml_dtypes

from concourse import bass, bacc, tile, mybir
from concourse.bass_utils import run_bass_kernel_spmd

P = 128
D = 64
NCORES = 8
ALPHA = 0.1
K_STEPS = 10

LAST_RESULT = None  # test harness reads exec_time_ns from here


class Cfg:
    def __init__(self, n_nodes, nb, chunk_slots):
        self.N = n_nodes
        self.NB = nb  # blocks per core
        self.SHARD = nb * P
        self.NPAD = NCORES * nb * P
        self.CHUNK = chunk_slots


FULL = Cfg(100000, 98, 224)


def _preprocess(cfg, x, edge_row, edge_col, edge_weight):
    N, NB, SHARD, NPAD = cfg.N, cfg.NB, cfg.SHARD, cfg.NPAD
    deg = np.bincount(edge_row, minlength=N)
    order = np.argsort(-deg, kind="stable").astype(np.int64)
    rows_sorted = np.concatenate([order, np.full(NPAD - N, -1, np.int64)])
    nblocks = NPAD // P

    # snake assignment of degree-sorted blocks to cores: slot s of core k gets
    # block 8s+k (s even) or 8s+(7-k) (s odd) -> per-slot degrees are near-equal
    # across cores and per-core totals balance.
    block_of = np.empty((NCORES, NB), np.int64)
    for s in range(NB):
        base = s * NCORES
        for k in range(NCORES):
            block_of[k, s] = base + (k if s % 2 == 0 else NCORES - 1 - k)

    blk_maxdeg = np.zeros(nblocks, np.int64)
    for b in range(nblocks):
        rows = rows_sorted[b * P : (b + 1) * P]
        real = rows[rows >= 0]
        blk_maxdeg[b] = deg[real].max() if len(real) else 0

    g_s = np.maximum(1, np.array([blk_maxdeg[block_of[:, s]].max() for s in range(NB)], np.int64))
    offs = np.zeros(NB + 1, np.int64)
    np.cumsum(g_s, out=offs[1:])
    total = int(offs[-1])

    # new id: core k, slot s, partition p -> k*SHARD + s*P + p ; old id rows_sorted[block]
    new_rows_old = np.empty(NPAD, np.int64)  # new position -> old id (-1 virtual)
    for k in range(NCORES):
        for s in range(NB):
            b = block_of[k, s]
            new_rows_old[k * SHARD + s * P : k * SHARD + s * P + P] = rows_sorted[b * P : (b + 1) * P]
    old_to_new = np.full(N, -1, np.int64)
    mask = new_rows_old >= 0
    old_to_new[new_rows_old[mask]] = np.nonzero(mask)[0]

    r_new = old_to_new[edge_row]
    c_new = old_to_new[edge_col].astype(np.int32)
    w = edge_weight.astype(ml_dtypes.bfloat16)

    # group edges by destination row (new id); rank within row
    eorder = np.argsort(r_new, kind="stable")
    r_s = r_new[eorder]
    c_s = c_new[eorder]
    w_s = w[eorder]
    row_counts = np.bincount(r_s, minlength=NPAD)
    row_starts = np.zeros(NPAD, np.int64)
    np.cumsum(row_counts[:-1], out=row_starts[1:])
    j_rank = np.arange(len(r_s)) - row_starts[r_s]

    core_e = r_s // SHARD
    s_e = (r_s % SHARD) // P
    p_e = r_s % P

    idx_ell = np.zeros((NCORES, P, total), np.int32)
    w_ell = np.zeros((NCORES, P, total), ml_dtypes.bfloat16)
    col_e = offs[s_e] + j_rank
    flat = p_e * total + col_e
    for k in range(NCORES):
        m = core_e == k
        idx_ell[k].reshape(-1)[flat[m]] = c_s[m]
        w_ell[k].reshape(-1)[flat[m]] = w_s[m]

    # x arranged per core: [P, NB, D], pre-scaled by alpha/(1-alpha)
    x_new = np.zeros((NPAD, D), np.float32)
    x_new[mask] = x[new_rows_old[mask]]
    scale = ALPHA / (1.0 - ALPHA)
    x_ell = np.empty((NCORES, P, NB, D), np.float32)
    for k in range(NCORES):
        x_ell[k] = (x_new[k * SHARD : (k + 1) * SHARD] * scale).reshape(NB, P, D).transpose(1, 0, 2)

    h0 = x_new.astype(ml_dtypes.bfloat16)

    # chunk slots greedily so each gather tile holds <= CHUNK slots
    chunks = []  # (slot_lo, slot_hi, off_lo, off_hi)
    s0 = 0
    while s0 < NB:
        s1 = s0 + 1
        while s1 < NB and offs[s1 + 1] - offs[s0] <= cfg.CHUNK:
            s1 += 1
        chunks.append((s0, s1, int(offs[s0]), int(offs[s1])))
        s0 = s1
    max_chunk = max(c[3] - c[2] for c in chunks)

    struct = (tuple(int(g) for g in g_s), tuple(chunks), max_chunk, total)
    return struct, idx_ell, w_ell, x_ell, h0, new_rows_old


def _build(cfg, struct):
    g_s, chunks, max_chunk, total = struct
    NB, SHARD, NPAD = cfg.NB, cfg.SHARD, cfg.NPAD
    offs = np.zeros(NB + 1, np.int64)
    np.cumsum(np.array(g_s), out=offs[1:])

    nc = bacc.Bacc("TRN2", target_bir_lowering=False, debug=False, num_devices=NCORES)
    bf16, f32, i32 = mybir.dt.bfloat16, mybir.dt.float32, mybir.dt.int32

    idx_in = nc.dram_tensor("idx", [P, total], i32, kind="ExternalInput")
    w_in = nc.dram_tensor("w", [P, total], bf16, kind="ExternalInput")
    x_in = nc.dram_tensor("x", [P, NB, D], f32, kind="ExternalInput")
    h0_in = nc.dram_tensor("h0", [NPAD, D], bf16, kind="ExternalInput")
    out_ext = nc.dram_tensor("out", [P, NB, D], f32, kind="ExternalOutput")

    tabA = nc.dram_tensor("tabA", [NPAD, D], bf16)
    tabB = nc.dram_tensor("tabB", [NPAD, D], bf16)
    sbA = nc.dram_tensor("sbA", [SHARD, D], bf16)
    sbB = nc.dram_tensor("sbB", [SHARD, D], bf16)

    with tile.TileContext(nc) as tc:
        with (
            tc.tile_pool(name="const", bufs=1) as cpool,
            tc.tile_pool(name="gath", bufs=2) as gpool,
            tc.tile_pool(name="red", bufs=2) as rpool,
            tc.tile_pool(name="hb", bufs=2) as hbpool,
        ):
            idx_t = cpool.tile([P, total], i32, tag="idx")
            w_t = cpool.tile([P, total], bf16, tag="w")
            x_t = cpool.tile([P, NB * D], f32, tag="x")
            nc.sync.dma_start(out=idx_t[:], in_=idx_in[:])
            nc.sync.dma_start(out=w_t[:], in_=w_in[:])
            nc.sync.dma_start(out=x_t[:], in_=x_in[:].rearrange("p b d -> p (b d)"))

            tables = [h0_in]
            for t in range(K_STEPS - 1):
                tables.append(tabA if t % 2 == 0 else tabB)

            for t in range(K_STEPS):
                src = tables[t]
                red = rpool.tile([P, NB * D], f32, tag="red")
                for (s0, s1, lo, hi) in chunks:
                    csz = hi - lo
                    gt = gpool.tile([P, max_chunk * D], bf16, tag="g")
                    nc.gpsimd.indirect_dma_start(
                        out=gt[:, : csz * D],
                        out_offset=None,
                        in_=src[:],
                        in_offset=bass.IndirectOffsetOnAxis(ap=idx_t[:, lo:hi], axis=0),
                    )
                    wb = w_t[:, lo:hi].unsqueeze(-1).to_broadcast([P, csz, D])
                    nc.vector.tensor_tensor(
                        out=gt[:, : csz * D].rearrange("p (g d) -> p g d", d=D),
                        in0=gt[:, : csz * D].rearrange("p (g d) -> p g d", d=D),
                        in1=wb,
                        op=mybir.AluOpType.mult,
                    )
                    for s in range(s0, s1):
                        g = g_s[s]
                        a = (int(offs[s]) - lo) * D
                        seg = gt[:, a : a + g * D]
                        seg_t = seg.rearrange("p (g d) -> p d g", d=D)
                        nc.vector.tensor_reduce(
                            out=red[:, s * D : (s + 1) * D],
                            in_=seg_t,
                            axis=mybir.AxisListType.X,
                            op=mybir.AluOpType.add,
                        )
                # h_{t+1} = 0.9 * (red + x/9) ; do add in place, scale on the cast/store
                nc.vector.tensor_tensor(
                    out=red[:], in0=red[:], in1=x_t[:], op=mybir.AluOpType.add
                )
                if t < K_STEPS - 1:
                    hb = hbpool.tile([P, NB * D], bf16, tag="hb")
                    nc.vector.tensor_scalar_mul(out=hb[:], in0=red[:], scalar1=1.0 - ALPHA)
                    sb = sbA if t % 2 == 0 else sbB
                    nc.sync.dma_start(
                        out=sb[:].rearrange("(s p) d -> p s d", p=P),
                        in_=hb[:].rearrange("p (s d) -> p s d", d=D),
                    )
                    nc.gpsimd.collective_compute(
                        "AllGather",
                        mybir.AluOpType.bypass,
                        replica_groups=[list(range(NCORES))],
                        ins=[sb.ap().opt()],
                        outs=[tables[t + 1].ap().opt()],
                    )
                else:
                    fin = hbpool.tile([P, NB * D], f32, tag="fin")
                    nc.vector.tensor_scalar_mul(out=fin[:], in0=red[:], scalar1=1.0 - ALPHA)
                    nc.sync.dma_start(
                        out=out_ext[:].rearrange("p b d -> p (b d)"), in_=fin[:]
                    )
    nc.compile()
    return nc


_BUILD_CACHE = {}


def _kernel_impl(cfg, x, edge_row, edge_col, edge_weight, trace=False):
    global LAST_RESULT
    struct, idx_ell, w_ell, x_ell, h0, new_rows_old = _preprocess(
        cfg, x, edge_row, edge_col, edge_weight
    )
    key = (cfg.N, struct[0], struct[1])
    if key not in _BUILD_CACHE:
        _BUILD_CACHE[key] = _build(cfg, struct)
    nc = _BUILD_CACHE[key]

    in_maps = [
        {"idx": idx_ell[k], "w": w_ell[k], "x": x_ell[k], "h0": h0}
        for k in range(NCORES)
    ]
    res = run_bass_kernel_spmd(nc, in_maps, core_ids=list(range(NCORES)), trace=trace)
    LAST_RESULT = res

    SHARD = cfg.SHARD
    full_new = np.empty((cfg.NPAD, D), np.float32)
    for k in range(NCORES):
        o = res.results[k]["out"]  # [P, NB, D]
        full_new[k * SHARD : (k + 1) * SHARD] = o.transpose(1, 0, 2).reshape(SHARD, D)
    out = np.empty((cfg.N, D), np.float32)
    mask = new_rows_old >= 0
    out[new_rows_old[mask]] = full_new[mask]
    return out


def kernel(x, edge_row, edge_col, edge_weight, _trace=False):
    x = np.asarray(x, dtype=np.float32)
    edge_row = np.asarray(edge_row, dtype=np.int32)
    edge_col = np.asarray(edge_col, dtype=np.int32)
    edge_weight = np.asarray(edge_weight, dtype=np.float32)
    return _kernel_impl(FULL, x, edge_row, edge_col, edge_weight, trace=_trace)


# revision 3
# speedup vs baseline: 23.8291x; 23.8291x over previous
"""APPNP propagation (10 iterations of h <- 0.9*A@h + 0.1*x) on 8 TRN2 NeuronCores.

Strategy (edge/row sharding + ELLPACK):
  - Nodes are permuted so rows are grouped by degree into blocks of 128; each
    core owns 98 blocks (12544 rows) chosen snake-wise so per-core edge counts
    balance. Per block, every row's neighbor list is padded to the block's max
    degree g, giving a regular [128, g] index/weight layout per block.
  - Per iteration, each core indirect-DMA-gathers the bf16 h rows of its
    edges' source nodes ([128, g, 64] tiles), multiplies by edge weights and
    segment-reduces on the vector engine, adds the residual, and AllGathers
    the bf16 h shards into the next iteration's gather table.
  - Output rows are written once (f32) on the last iteration and unpermuted
    on the host.
"""

import sys

sys.path.insert(0, "/opt/trn_rl_repo")

import numpy as np
import ml_dtypes

from concourse import bass, bacc, tile, mybir
from concourse.bass_utils import run_bass_kernel_spmd

P = 128
D = 64
NCORES = 8
ALPHA = 0.1
K_STEPS = 10

LAST_RESULT = None  # test harness reads exec_time_ns from here


class Cfg:
    def __init__(self, n_nodes, nb, chunk_slots):
        self.N = n_nodes
        self.NB = nb  # blocks per core
        self.SHARD = nb * P
        self.NPAD = NCORES * nb * P
        self.CHUNK = chunk_slots


FULL = Cfg(100000, 98, 224)


def _preprocess(cfg, x, edge_row, edge_col, edge_weight):
    N, NB, SHARD, NPAD = cfg.N, cfg.NB, cfg.SHARD, cfg.NPAD
    deg = np.bincount(edge_row, minlength=N)
    order = np.argsort(-deg, kind="stable").astype(np.int64)
    rows_sorted = np.concatenate([order, np.full(NPAD - N, -1, np.int64)])
    nblocks = NPAD // P

    # snake assignment of degree-sorted blocks to cores: slot s of core k gets
    # block 8s+k (s even) or 8s+(7-k) (s odd) -> per-slot degrees are near-equal
    # across cores and per-core totals balance.
    block_of = np.empty((NCORES, NB), np.int64)
    for s in range(NB):
        base = s * NCORES
        for k in range(NCORES):
            block_of[k, s] = base + (k if s % 2 == 0 else NCORES - 1 - k)

    blk_maxdeg = np.zeros(nblocks, np.int64)
    for b in range(nblocks):
        rows = rows_sorted[b * P : (b + 1) * P]
        real = rows[rows >= 0]
        blk_maxdeg[b] = deg[real].max() if len(real) else 0

    g_s = np.maximum(1, np.array([blk_maxdeg[block_of[:, s]].max() for s in range(NB)], np.int64))
    offs = np.zeros(NB + 1, np.int64)
    np.cumsum(g_s, out=offs[1:])
    total = int(offs[-1])

    # new id: core k, slot s, partition p -> k*SHARD + s*P + p ; old id rows_sorted[block]
    new_rows_old = np.empty(NPAD, np.int64)  # new position -> old id (-1 virtual)
    for k in range(NCORES):
        for s in range(NB):
            b = block_of[k, s]
            new_rows_old[k * SHARD + s * P : k * SHARD + s * P + P] = rows_sorted[b * P : (b + 1) * P]
    old_to_new = np.full(N, -1, np.int64)
    mask = new_rows_old >= 0
    old_to_new[new_rows_old[mask]] = np.nonzero(mask)[0]

    r_new = old_to_new[edge_row]
    c_new = old_to_new[edge_col].astype(np.int32)
    w = edge_weight.astype(ml_dtypes.bfloat16)

    # group edges by destination row (new id); rank within row
    eorder = np.argsort(r_new, kind="stable")
    r_s = r_new[eorder]
    c_s = c_new[eorder]
    w_s = w[eorder]
    row_counts = np.bincount(r_s, minlength=NPAD)
    row_starts = np.zeros(NPAD, np.int64)
    np.cumsum(row_counts[:-1], out=row_starts[1:])
    j_rank = np.arange(len(r_s)) - row_starts[r_s]

    core_e = r_s // SHARD
    s_e = (r_s % SHARD) // P
    p_e = r_s % P

    idx_ell = np.zeros((NCORES, P, total), np.int32)
    w_ell = np.zeros((NCORES, P, total), ml_dtypes.bfloat16)
    col_e = offs[s_e] + j_rank
    flat = p_e * total + col_e
    for k in range(NCORES):
        m = core_e == k
        idx_ell[k].reshape(-1)[flat[m]] = c_s[m]
        w_ell[k].reshape(-1)[flat[m]] = w_s[m]

    # x arranged per core: [P, NB, D], pre-scaled by alpha/(1-alpha)
    x_new = np.zeros((NPAD, D), np.float32)
    x_new[mask] = x[new_rows_old[mask]]
    scale = ALPHA / (1.0 - ALPHA)
    x_ell = np.empty((NCORES, P, NB, D), np.float32)
    for k in range(NCORES):
        x_ell[k] = (x_new[k * SHARD : (k + 1) * SHARD] * scale).reshape(NB, P, D).transpose(1, 0, 2)

    h0 = x_new.astype(ml_dtypes.bfloat16)

    # chunk slots greedily so each gather tile holds <= CHUNK slots
    chunks = []  # (slot_lo, slot_hi, off_lo, off_hi)
    s0 = 0
    while s0 < NB:
        s1 = s0 + 1
        while s1 < NB and offs[s1 + 1] - offs[s0] <= cfg.CHUNK:
            s1 += 1
        chunks.append((s0, s1, int(offs[s0]), int(offs[s1])))
        s0 = s1
    max_chunk = max(c[3] - c[2] for c in chunks)

    struct = (tuple(int(g) for g in g_s), tuple(chunks), max_chunk, total)
    return struct, idx_ell, w_ell, x_ell, h0, new_rows_old


def _build(cfg, struct):
    g_s, chunks, max_chunk, total = struct
    NB, SHARD, NPAD = cfg.NB, cfg.SHARD, cfg.NPAD
    offs = np.zeros(NB + 1, np.int64)
    np.cumsum(np.array(g_s), out=offs[1:])

    nc = bacc.Bacc("TRN2", target_bir_lowering=False, debug=False, num_devices=NCORES)
    bf16, f32, i32 = mybir.dt.bfloat16, mybir.dt.float32, mybir.dt.int32

    idx_in = nc.dram_tensor("idx", [P, total], i32, kind="ExternalInput")
    w_in = nc.dram_tensor("w", [P, total], bf16, kind="ExternalInput")
    x_in = nc.dram_tensor("x", [P, NB, D], f32, kind="ExternalInput")
    h0_in = nc.dram_tensor("h0", [NPAD, D], bf16, kind="ExternalInput")
    out_ext = nc.dram_tensor("out", [P, NB, D], f32, kind="ExternalOutput")

    tabA = nc.dram_tensor("tabA", [NPAD, D], bf16)
    tabB = nc.dram_tensor("tabB", [NPAD, D], bf16)
    sbA = nc.dram_tensor("sbA", [SHARD, D], bf16)
    sbB = nc.dram_tensor("sbB", [SHARD, D], bf16)

    with tile.TileContext(nc) as tc:
        with (
            tc.tile_pool(name="const", bufs=1) as cpool,
            tc.tile_pool(name="gath", bufs=2) as gpool,
            tc.tile_pool(name="red", bufs=1) as rpool,
            tc.tile_pool(name="hb", bufs=2) as hbpool,
        ):
            idx_t = cpool.tile([P, total], i32, tag="idx")
            w_t = cpool.tile([P, total], bf16, tag="w")
            x_t = cpool.tile([P, NB * D], f32, tag="x")
            nc.sync.dma_start(out=idx_t[:], in_=idx_in[:])
            nc.sync.dma_start(out=w_t[:], in_=w_in[:])
            nc.sync.dma_start(out=x_t[:], in_=x_in[:].rearrange("p b d -> p (b d)"))

            tables = [h0_in]
            for t in range(K_STEPS - 1):
                tables.append(tabA if t % 2 == 0 else tabB)

            for t in range(K_STEPS):
                src = tables[t]
                red = rpool.tile([P, NB * D], f32, tag="red")
                for (s0, s1, lo, hi) in chunks:
                    csz = hi - lo
                    gt = gpool.tile([P, max_chunk * D], bf16, tag="g")
                    nc.gpsimd.indirect_dma_start(
                        out=gt[:, : csz * D],
                        out_offset=None,
                        in_=src[:],
                        in_offset=bass.IndirectOffsetOnAxis(ap=idx_t[:, lo:hi], axis=0),
                    )
                    wb = w_t[:, lo:hi].unsqueeze(-1).to_broadcast([P, csz, D])
                    nc.vector.tensor_tensor(
                        out=gt[:, : csz * D].rearrange("p (g d) -> p g d", d=D),
                        in0=gt[:, : csz * D].rearrange("p (g d) -> p g d", d=D),
                        in1=wb,
                        op=mybir.AluOpType.mult,
                    )
                    for s in range(s0, s1):
                        g = g_s[s]
                        a = (int(offs[s]) - lo) * D
                        seg = gt[:, a : a + g * D]
                        seg_t = seg.rearrange("p (g d) -> p d g", d=D)
                        nc.vector.tensor_reduce(
                            out=red[:, s * D : (s + 1) * D],
                            in_=seg_t,
                            axis=mybir.AxisListType.X,
                            op=mybir.AluOpType.add,
                        )
                # h_{t+1} = 0.9 * (red + x/9) ; do add in place, scale on the cast/store
                nc.vector.tensor_tensor(
                    out=red[:], in0=red[:], in1=x_t[:], op=mybir.AluOpType.add
                )
                if t < K_STEPS - 1:
                    hb = hbpool.tile([P, NB * D], bf16, tag="hb")
                    nc.vector.tensor_scalar_mul(out=hb[:], in0=red[:], scalar1=1.0 - ALPHA)
                    sb = sbA if t % 2 == 0 else sbB
                    nc.sync.dma_start(
                        out=sb[:].rearrange("(s p) d -> p s d", p=P),
                        in_=hb[:].rearrange("p (s d) -> p s d", d=D),
                    )
                    nc.gpsimd.collective_compute(
                        "AllGather",
                        mybir.AluOpType.bypass,
                        replica_groups=[list(range(NCORES))],
                        ins=[sb.ap().opt()],
                        outs=[tables[t + 1].ap().opt()],
                    )
                else:
                    fin = hbpool.tile([P, NB * D], f32, tag="hb")
                    nc.vector.tensor_scalar_mul(out=fin[:], in0=red[:], scalar1=1.0 - ALPHA)
                    nc.sync.dma_start(
                        out=out_ext[:].rearrange("p b d -> p (b d)"), in_=fin[:]
                    )
    nc.compile()
    return nc


_BUILD_CACHE = {}


def _kernel_impl(cfg, x, edge_row, edge_col, edge_weight, trace=False):
    global LAST_RESULT
    struct, idx_ell, w_ell, x_ell, h0, new_rows_old = _preprocess(
        cfg, x, edge_row, edge_col, edge_weight
    )
    key = (cfg.N, struct[0], struct[1])
    if key not in _BUILD_CACHE:
        _BUILD_CACHE[key] = _build(cfg, struct)
    nc = _BUILD_CACHE[key]

    in_maps = [
        {"idx": idx_ell[k], "w": w_ell[k], "x": x_ell[k], "h0": h0}
        for k in range(NCORES)
    ]
    res = run_bass_kernel_spmd(nc, in_maps, core_ids=list(range(NCORES)), trace=trace)
    LAST_RESULT = res

    SHARD = cfg.SHARD
    full_new = np.empty((cfg.NPAD, D), np.float32)
    for k in range(NCORES):
        o = np.asarray(res.results[k]["out"]).reshape(P, cfg.NB, D)
        full_new[k * SHARD : (k + 1) * SHARD] = o.transpose(1, 0, 2).reshape(SHARD, D)
    out = np.empty((cfg.N, D), np.float32)
    mask = new_rows_old >= 0
    out[new_rows_old[mask]] = full_new[mask]
    return out


def kernel(x, edge_row, edge_col, edge_weight, _trace=False):
    x = np.asarray(x, dtype=np.float32)
    edge_row = np.asarray(edge_row, dtype=np.int32)
    edge_col = np.asarray(edge_col, dtype=np.int32)
    edge_weight = np.asarray(edge_weight, dtype=np.float32)
    return _kernel_impl(FULL, x, edge_row, edge_col, edge_weight, trace=_trace)
